# revision 1
# baseline (speedup 1.0000x reference)
"""Trainium2 Bass kernel for nn_BinaryPathEncoder.

Math: out[n] = prod_{k} W_{b_k(pos_n)}^T  (product over the binary digits of
pos_n below its leading 1; W_0/W_1 = expm(herm_b), pad -> identity).

Let G_b = W_b^T = expm(-herm_b), M(h) = G_{b_0(h)} @ G_{b_1(h)} @ ... .
Split pos = hi*256 + lo:
  hi >= 1:  out = A8(lo) @ B(hi)   where A8(m) = 8-bit all-valid product,
                                          B(h) = M(h)  (h < 256)
  hi == 0:  out = M(pos) = B[pos]  = I @ B[pos]

Device (SPMD, identical program on 8 cores; per-core data differs):
  - expm via scaling-squaring Taylor (matmul-only, no solves)
  - builds A2/A4 doubling tables, the 256-entry B table (SBUF), and the
    257-entry stationary table STAT = [A8^T entries; identity] (DRAM)
  - position loop: per block of 4 positions sharing one stationary entry:
    stationary staged by dynamic-offset DMA from STAT, 4 moving operands
    staged from the SBUF B table, 4 matmuls into one PSUM bank, DVE evac,
    batched output DMA.
Host: computes lo/hi, packs positions into blocks (padded), distributes
blocks over cores, scatters results back to original order.
"""

import contextlib
import os

import numpy as np

import concourse.bass as bass
import concourse.bacc as bacc
import concourse.mybir as mybir
import concourse.tile as tile
import concourse.tile_utils as tile_utils
tile_utils.max_sbuf_usage = 206 * 1024
from concourse.bass_utils import run_bass_kernel_spmd
from concourse.masks import make_identity

FP = mybir.dt.float32
I32 = mybir.dt.int32
P = 128
NCORES = 8
S_EXP = 5          # scaling-squaring: X = -H / 2^S_EXP
ORDER = 18         # Taylor order
NB = 256           # B-table entries (hi < 256)
IDENT_ENTRY = 256  # stationary-table entry holding the identity

# mover engine per position-in-block: how the 4 moving operands get staged
#   "sync"/"gpsimd": dyn-offset DMA from the DRAM B copy
#   "vector"/"scalar": dyn-offset compute-engine copy from the SBUF B table
#   "gpsimd_sb": gpsimd SBUF->SBUF dma from the SBUF B table
MOVERS = tuple(os.environ.get("MOVERS", "sync,gpsimd,sync,gpsimd").split(","))
NEED_BDRAM = any(m in ("sync", "gpsimd") for m in MOVERS)

_prog_cache = {}
_last_ctx = None


def _mm(nc, out, lhsT, rhs):
    nc.tensor.matmul(out, lhsT=lhsT, rhs=rhs, start=True, stop=True)


def _build_expm(nc, consts, psump, scratch, praw, ident):
    """Return (G, GT) tile pairs: G_b = expm(-H_b), GT_b = G_b^T."""
    Gs, GTs = [], []
    for b in range(2):
        pb = praw[:, b, :]
        ps_t = psump.tile([P, 512], FP, tag="pos")
        nc.tensor.transpose(out=ps_t[:, :P], in_=pb, identity=ident[:])
        xt = consts.tile([P, P], FP, tag=f"xt{b}")
        # XT = (P - P^T)/2^s ;  lhsT=XT gives out = (-H/2^s) @ rhs since H^T=-H
        nc.vector.tensor_tensor(
            out=xt[:], in0=pb, in1=ps_t[:, :P], op=mybir.AluOpType.subtract
        )
        nc.vector.tensor_scalar_mul(xt[:], xt[:], 1.0 / (1 << S_EXP))

        t_cur = scratch.tile([P, P], FP, tag="tay")
        nc.vector.tensor_copy(t_cur[:], ident[:])
        for k in range(ORDER, 0, -1):
            ps = psump.tile([P, 512], FP, tag="pos")
            _mm(nc, ps[:, :P], xt[:], t_cur[:])
            t_nxt = scratch.tile([P, P], FP, tag="tay")
            nc.vector.tensor_scalar_mul(t_nxt[:], ps[:, :P], 1.0 / k)
            nc.vector.tensor_add(t_nxt[:], t_nxt[:], ident[:])
            t_cur = t_nxt
        # U = T^T
        ps_u = psump.tile([P, 512], FP, tag="pos")
        nc.tensor.transpose(out=ps_u[:, :P], in_=t_cur[:], identity=ident[:])
        u_cur = scratch.tile([P, P], FP, tag="tayu")
        nc.vector.tensor_copy(u_cur[:], ps_u[:, :P])
        for _ in range(S_EXP):
            ps1 = psump.tile([P, 512], FP, tag="pos")
            ps2 = psump.tile([P, 512], FP, tag="pos")
            _mm(nc, ps1[:, :P], u_cur[:], t_cur[:])   # T' = T @ T
            _mm(nc, ps2[:, :P], t_cur[:], u_cur[:])   # U' = (T@T)^T
            t_cur = scratch.tile([P, P], FP, tag="tay")
            u_cur = scratch.tile([P, P], FP, tag="tayu")
            nc.vector.tensor_copy(t_cur[:], ps1[:, :P])
            nc.vector.tensor_copy(u_cur[:], ps2[:, :P])
        g = consts.tile([P, P], FP, tag=f"g{b}")
        gt = consts.tile([P, P], FP, tag=f"gt{b}")
        nc.vector.tensor_copy(g[:], t_cur[:])
        nc.vector.tensor_copy(gt[:], u_cur[:])
        Gs.append(g)
        GTs.append(gt)
    return Gs, GTs


def build_program(n16, n4):
    nblk = n16 + n4
    nslots = n16 * 16 + n4 * 4
    nc = bacc.Bacc("TRN2", target_bir_lowering=False, debug=False,
                   num_devices=NCORES)
    praw_d = nc.dram_tensor("praw", [2, P, P], FP, kind="ExternalInput")
    sioff_d = nc.dram_tensor("sioff", [1, nblk], I32, kind="ExternalInput")
    bidx_d = nc.dram_tensor("bidx", [P, nslots], I32, kind="ExternalInput")
    out_d = nc.dram_tensor("out", [P, nslots * P], FP, kind="ExternalOutput")
    stat_d = nc.dram_tensor("stat", [(NB + 1) * P, P], FP)
    bdram_d = nc.dram_tensor("bdram", [NB * P, P], FP)

    with tile.TileContext(nc) as tc:
        with (
            tc.tile_pool(name="consts", bufs=1) as consts,
            tc.tile_pool(name="scratch", bufs=2) as scratch,
            tc.tile_pool(name="atab", bufs=1) as atab,
            tc.tile_pool(name="btab", bufs=1) as btabp,
            tc.tile_pool(name="sstage", bufs=2) as sstagep,
            tc.tile_pool(name="stage", bufs=int(os.environ.get("STAGE_BUFS", "8"))) as stagep,
            tc.tile_pool(name="mv16", bufs=int(os.environ.get("MV16_BUFS", "2"))) as mv16p,
            tc.tile_pool(name="mv", bufs=int(os.environ.get("MV_BUFS", "4"))) as mvp,
            tc.tile_pool(name="obuf", bufs=int(os.environ.get("OBUF_BUFS", "2"))) as obufp,
            tc.tile_pool(name="psum", bufs=int(os.environ.get("PSUM_BUFS", "8")), space="PSUM") as psump,
        ):
            ident = consts.tile([P, P], FP, tag="ident")
            make_identity(nc, ident[:])
            praw = consts.tile([P, 2, P], FP, tag="praw")
            nc.sync.dma_start(praw[:], praw_d[:].rearrange("p r c -> r p c"))
            sioff = consts.tile([1, nblk], I32, tag="sioff")
            bidx = consts.tile([P, nslots], I32, tag="bidx")
            nc.sync.dma_start(sioff[:], sioff_d[:])
            nc.sync.dma_start(bidx[:], bidx_d[:])

            # ---- phase A: primitives ----
            G, GT = _build_expm(nc, consts, psump, scratch, praw, ident)

            # ---- phase B: A2/A2T/A4/A4T doubling tables ----
            a2 = atab.tile([P, 4, P], FP, tag="a2")
            a2t = atab.tile([P, 4, P], FP, tag="a2t")
            for m in range(4):
                ps = psump.tile([P, 512], FP, tag="pos")
                _mm(nc, ps[:, :P], GT[m & 1][:], G[m >> 1][:])   # A2[m]
                nc.vector.tensor_copy(a2[:, m, :], ps[:, :P])
                ps2 = psump.tile([P, 512], FP, tag="pos")
                _mm(nc, ps2[:, :P], G[m >> 1][:], GT[m & 1][:])  # A2T[m]
                nc.vector.tensor_copy(a2t[:, m, :], ps2[:, :P])
            a4 = atab.tile([P, 16, P], FP, tag="a4")
            a4t = atab.tile([P, 16, P], FP, tag="a4t")
            a2f = a2[:].rearrange("r m c -> r (m c)")
            a2tf = a2t[:].rearrange("r m c -> r (m c)")
            for a in range(4):
                ps = psump.tile([P, 512], FP, tag="pos")
                _mm(nc, ps[:], a2t[:, a, :], a2f)        # A4[a+4b] over b
                for b2 in range(4):
                    nc.vector.tensor_copy(
                        a4[:, a + 4 * b2, :], ps[:, b2 * P : (b2 + 1) * P]
                    )
                # A4T[m] = A2T[m>>2] @ A2T[m&3]; fix g=m>>2: m = 4g+b contiguous
                ps2 = psump.tile([P, 512], FP, tag="pos")
                _mm(nc, ps2[:], a2[:, a, :], a2tf)
                nc.vector.tensor_copy(
                    a4t[:, 4 * a : 4 * a + 4, :].rearrange("r m c -> r (m c)"),
                    ps2[:],
                )

            # ---- phase C: S^T table (A8^T) -> stat_d[0:256], identity -> [256]
            a4tf = a4t[:].rearrange("r m c -> r (m c)")
            stat_v = stat_d[:].rearrange("(e r) c -> r e c", r=P)
            for g in range(16):
                for q in range(4):
                    sst = sstagep.tile([P, 4, P], FP, tag="sst")
                    ps = psump.tile([P, 512], FP, tag="pos")
                    # S^T[16g + (4q+j)] = A4T[g] @ A4T[4q+j], j=0..3
                    _mm(nc, ps[:], a4[:, g, :], a4tf[:, q * 512 : (q + 1) * 512])
                    nc.vector.tensor_copy(
                        sst[:].rearrange("r m c -> r (m c)"), ps[:]
                    )
                    nc.sync.dma_start(
                        stat_v[:, 16 * g + 4 * q : 16 * g + 4 * q + 4, :],
                        sst[:],
                    )
            nc.sync.dma_start(stat_v[:, NB : NB + 1, :], ident[:, None, :])

            # ---- phase D: B table (SBUF, optionally DRAM copy) ----
            btab = btabp.tile([P, NB, P], FP, tag="btab")
            nc.vector.tensor_copy(btab[:, 0, :], ident[:])
            nc.vector.tensor_copy(btab[:, 1, :], ident[:])
            for lvl in range(1, 8):
                p0, p1 = 1 << (lvl - 1), 1 << lvl
                for b in range(2):
                    for c0 in range(p0, p1, 4):
                        npar = min(4, p1 - c0)
                        ps = psump.tile([P, 512], FP, tag="pos")
                        _mm(
                            nc,
                            ps[:, : npar * P],
                            GT[b][:],
                            btab[:, c0 : c0 + npar, :].rearrange(
                                "r m c -> r (m c)"
                            ),
                        )
                        for j in range(npar):
                            nc.vector.tensor_copy(
                                btab[:, 2 * (c0 + j) + b, :],
                                ps[:, j * P : (j + 1) * P],
                            )
            nc.sync.dma_start(
                bdram_d[:].rearrange("(e r) c -> r e c", r=P), btab[:]
            )

            # ---- phase E: position loop ----
            # B16 blocks: 1 stationary stage (SWDGE dyn DMA) + 1 indirect
            # gather of 16 moving entries + 4 matmuls N=512 + 4 evacs.
            # B4 blocks: same with 4 entries / 1 matmul / 1 evac.
            with (
                nc.gpsimd.register("rg") as rg,
                nc.scalar.register("ra") as ra,
                nc.sync.register("rs") as rs,
            ):
                def do_block(blk, s0, size, ob, obase):
                    st = stagep.tile([P, P], FP, tag="st")
                    nc.sync.reg_load(rs, sioff[0:1, blk : blk + 1])
                    so = nc.sync.snap(rs)
                    nc.sync.dma_start(st[:], stat_d[bass.ds(so, P), :])
                    if size == 16:
                        mv = mv16p.tile([P, 16, P], FP, tag="mv16")
                    else:
                        mv = mvp.tile([P, 4, P], FP, tag="mv4")
                    for j in range(size):
                        if j % 2 == 0:
                            eng, reg = nc.gpsimd, rg
                        else:
                            eng, reg = nc.scalar, ra
                        eng.reg_load(reg, bidx[0:1, s0 + j : s0 + j + 1])
                        bo = eng.snap(reg)
                        eng.dma_start(mv[:, j, :], bdram_d[bass.ds(bo, P), :])
                    for q in range(size // 4):
                        ps = psump.tile([P, 512], FP, tag="pos")
                        _mm(
                            nc,
                            ps[:],
                            st[:],
                            mv[:, 4 * q : 4 * q + 4, :].rearrange(
                                "r m c -> r (m c)"
                            ),
                        )
                        nc.vector.tensor_copy(
                            ob[:, obase + 4 * q * P : obase + (4 * q + 4) * P],
                            ps[:],
                        )

                for b in range(n16):
                    ob = obufp.tile([P, 16 * P], FP, tag="ob")
                    do_block(b, b * 16, 16, ob, 0)
                    nc.sync.dma_start(
                        out_d[:, b * 16 * P : (b + 1) * 16 * P], ob[:]
                    )
                base16 = n16 * 16
                for c0 in range(0, n4, 4):
                    nbi = min(4, n4 - c0)
                    ob = obufp.tile([P, 16 * P], FP, tag="ob")
                    for k in range(nbi):
                        blk = n16 + c0 + k
                        do_block(blk, base16 + (c0 + k) * 4, 4, ob, k * 4 * P)
                    nc.sync.dma_start(
                        out_d[
                            :,
                            (base16 + c0 * 4) * P : (base16 + (c0 + nbi) * 4) * P,
                        ],
                        ob[:, : nbi * 4 * P],
                    )
    nc.compile()
    return nc


def _plan_blocks(unique):
    """Pack positions into 16-blocks and 4-blocks sharing a stationary entry.

    Returns (blocks16, blocks4) where each block is (ent, [bents...]) with
    bents padded with -1 markers replaced by 0 later, plus member position
    indices for slot mapping.
    """
    n = unique.shape[0]
    lo = unique & 255
    hi = unique >> 8
    ent = np.where(hi > 0, lo, IDENT_ENTRY)
    bent = np.where(hi > 0, hi, unique)  # hi==0 -> out = I @ B[pos]
    order = np.argsort(ent, kind="stable")
    es = ent[order]
    bounds = np.flatnonzero(np.r_[True, es[1:] != es[:-1], True])

    blocks16, blocks4 = [], []
    for s, e in zip(bounds[:-1], bounds[1:]):
        idxs = order[s:e]
        v = int(es[s])
        g = len(idxs)
        q0 = 0
        while g - q0 >= 16:
            blocks16.append((v, idxs[q0 : q0 + 16]))
            q0 += 16
        while q0 < g:
            blocks4.append((v, idxs[q0 : q0 + 4]))
            q0 += 4
    return blocks16, blocks4, bent


def kernel(unique, primitives_raw, identity=None, **_):
    unique = np.asarray(unique)
    praw = np.ascontiguousarray(np.asarray(primitives_raw, np.float32))

    blocks16, blocks4, bent = _plan_blocks(unique.astype(np.int64))
    n16 = -(-len(blocks16) // NCORES)
    n4 = -(-len(blocks4) // NCORES)
    while len(blocks16) < NCORES * n16:
        blocks16.append((IDENT_ENTRY, np.empty(0, np.int64)))
    while len(blocks4) < NCORES * n4:
        blocks4.append((IDENT_ENTRY, np.empty(0, np.int64)))
    nslots = n16 * 16 + n4 * 4

    # per-core inputs + slot mapping
    slot_of_pos = np.zeros(unique.shape[0], np.int64)
    sioff = np.zeros((NCORES, n16 + n4), np.int32)
    bidx = np.zeros((NCORES, P, nslots), np.int32)
    rows = np.arange(P, dtype=np.int32)
    for i, (v, mem) in enumerate(blocks16):
        c, k = divmod(i, n16)
        sioff[c, k] = v * P
        for j, pidx in enumerate(mem):
            bidx[c, :, k * 16 + j] = int(bent[pidx]) * P + rows
            slot_of_pos[pidx] = c * nslots + k * 16 + j
        for j in range(len(mem), 16):
            bidx[c, :, k * 16 + j] = rows
    for i, (v, mem) in enumerate(blocks4):
        c, k = divmod(i, n4)
        sioff[c, n16 + k] = v * P
        base = n16 * 16 + k * 4
        for j, pidx in enumerate(mem):
            bidx[c, :, base + j] = int(bent[pidx]) * P + rows
            slot_of_pos[pidx] = c * nslots + base + j
        for j in range(len(mem), 4):
            bidx[c, :, base + j] = rows

    key = (n16, n4)
    if key not in _prog_cache:
        _prog_cache[key] = build_program(n16, n4)
    nc = _prog_cache[key]

    in_maps = [
        {
            "praw": praw,
            "sioff": np.ascontiguousarray(sioff[c].reshape(1, -1)),
            "bidx": np.ascontiguousarray(bidx[c]),
        }
        for c in range(NCORES)
    ]
    global _last_ctx
    _last_ctx = (nc, in_maps)
    res = run_bass_kernel_spmd(nc, in_maps, list(range(NCORES)))
    outs = np.concatenate(
        [
            res.results[c]["out"]
            .reshape(P, nslots, P)
            .transpose(1, 0, 2)
            for c in range(NCORES)
        ],
        axis=0,
    )
    return np.ascontiguousarray(outs[slot_of_pos]).astype(np.float32)


if __name__ == "__main__":
    rng = np.random.default_rng(0)
    u = rng.integers(1, 65536, 64).astype(np.int32)
    pr = rng.random((2, P, P), np.float32)
    o = kernel(u, pr)
    print(o.shape, o.dtype)



# revision 8
# speedup vs baseline: 2.4660x; 2.4660x over previous
"""Trainium2 Bass kernel for nn_BinaryPathEncoder.

Math: out[n] = prod_k W_{b_k(pos_n)}^T (product over the binary digits of
pos_n below its leading 1; W_0/W_1 = expm(herm_b), pad -> identity).

Let G_b = W_b^T = expm(-herm_b), M(h) = G_{b_0(h)} @ G_{b_1(h)} @ ...
Split pos = hi*256 + lo:
  hi >= 1:  out = A8(lo) @ M(hi)   (8 low bits all valid)
  hi == 0:  out = I @ M(pos)
Tables (per core, identical SPMD program):
  - G via scaling-squaring Taylor in fp32
  - A2/A4/A4T doubling tables in fp32
  - stat[lo] = A8(lo)^T = A4T[lo>>4] @ A4T[lo&15]: fp32r matmuls -> bf16 DRAM
  - btab[h] = M(h): M(1..15) in fp32 from G chains; M(16q+m) = A4(m) @ M(q)
    via fp32r matmuls; all stored bf16 in SBUF [P, 16, 16, P] (e = 16q+m)
Position loop: blocks of 16 slots; stationary staged by dyn-offset DMA from
the DRAM stat table; moving operands read DIRECTLY from the SBUF btab via
per-matmul dynamic (register) offsets -- no gather, N=128 bf16 matmuls.
PSUM [P,1024] groups evacuated fp32->bf16 alternately on vector/scalar;
bf16 outputs DMA'd out; host converts to fp32 and scatters to input order.
"""

import contextlib
import os

import numpy as np

import concourse.bass as bass
import concourse.bacc as bacc
import concourse.mybir as mybir
import concourse.tile as tile
import concourse.tile_utils as tile_utils
tile_utils.max_sbuf_usage = 206 * 1024
from concourse.bass_utils import run_bass_kernel_spmd
from concourse.masks import make_identity

FP = mybir.dt.float32
FR = mybir.dt.float32r
BF = mybir.dt.bfloat16
I32 = mybir.dt.int32
P = 128
NCORES = 8
S_EXP = 5          # scaling-squaring: X = -H / 2^S_EXP
ORDER = 12         # Taylor order (||H||~37 -> tail ~1e-8)
NB = 256           # table entries
IDENT_ENTRY = 256  # stationary-table entry holding the identity

_prog_cache = {}
_last_ctx = None


def _mm(nc, out, lhsT, rhs):
    nc.tensor.matmul(out, lhsT=lhsT, rhs=rhs, start=True, stop=True)





def _build_expm(nc, consts, psB, scratch, praw, ident):
    """Return (G, GT) fp32 tile pairs: G_b = expm(-H_b), GT_b = G_b^T.

    Interleaves the b=0/b=1 chains to hide serial latency. Taylor recurrence
    T <- I + (X/k) @ T with pre-scaled copies of X^T, identity added on DVE.
    """
    xtj, t, u = {}, {}, {}
    for b in range(2):
        pb = praw[:, b, :]
        ps_t = psB.tile([P, 512], FP, tag="psb")
        nc.tensor.transpose(out=ps_t[:, :P], in_=pb, identity=ident[:])
        xt = scratch.tile([P, P], FP, tag=f"xt{b}")
        nc.vector.tensor_tensor(
            out=xt[:], in0=pb, in1=ps_t[:, :P], op=mybir.AluOpType.subtract
        )
        xj = consts.tile([P, ORDER, P], FP, tag=f"xtj{b}")
        for k in range(1, ORDER + 1):
            s = 1.0 / ((1 << S_EXP) * k)
            if k % 2 == 0:
                nc.vector.tensor_scalar_mul(xj[:, k - 1, :], xt[:], s)
            else:
                nc.scalar.mul(xj[:, k - 1, :], xt[:], s)
        xtj[b] = xj
        tc0 = scratch.tile([P, P], FP, tag=f"tay{b}")
        nc.vector.tensor_copy(tc0[:], ident[:])
        t[b] = tc0
    for k in range(ORDER, 0, -1):
        for b in range(2):
            ps = psB.tile([P, 512], FP, tag="psb")
            _mm(nc, ps[:, :P], xtj[b][:, k - 1, :], t[b][:])
            tn = scratch.tile([P, P], FP, tag=f"tay{b}")
            nc.vector.tensor_tensor(
                out=tn[:], in0=ps[:, :P], in1=ident[:], op=mybir.AluOpType.add
            )
            t[b] = tn
    for b in range(2):
        ps_u = psB.tile([P, 512], FP, tag="psb")
        nc.tensor.transpose(out=ps_u[:, :P], in_=t[b][:], identity=ident[:])
        uc = scratch.tile([P, P], FP, tag=f"tayu{b}")
        nc.scalar.copy(uc[:], ps_u[:, :P])
        u[b] = uc
    for _ in range(S_EXP):
        for b in range(2):
            ps1 = psB.tile([P, 512], FP, tag="psb")
            ps2 = psB.tile([P, 512], FP, tag="psb")
            _mm(nc, ps1[:, :P], u[b][:], t[b][:])   # T' = T @ T
            _mm(nc, ps2[:, :P], t[b][:], u[b][:])   # U' = (T@T)^T
            tn = scratch.tile([P, P], FP, tag=f"tay{b}")
            un = scratch.tile([P, P], FP, tag=f"tayu{b}")
            nc.vector.tensor_copy(tn[:], ps1[:, :P])
            nc.scalar.copy(un[:], ps2[:, :P])
            t[b], u[b] = tn, un
    Gs, GTs = [], []
    for b in range(2):
        g = consts.tile([P, P], FP, tag=f"g{b}")
        gt = consts.tile([P, P], FP, tag=f"gt{b}")
        nc.vector.tensor_copy(g[:], t[b][:])
        nc.scalar.copy(gt[:], u[b][:])
        Gs.append(g)
        GTs.append(gt)
    return Gs, GTs


def build_program(n16, n4):
    assert n4 % 4 == 0
    nblk = n16 + n4
    nslots = n16 * 16 + n4 * 4
    nsb = n16 + n4 // 4          # superblocks of 16 slots
    nc = bacc.Bacc("TRN2", target_bir_lowering=False, debug=False,
                   num_devices=NCORES)
    praw_d = nc.dram_tensor("praw", [2, P, P], FP, kind="ExternalInput")
    sioff_d = nc.dram_tensor("sioff", [1, nblk], I32, kind="ExternalInput")
    bidx_d = nc.dram_tensor("bidx", [1, nslots], I32, kind="ExternalInput")
    out_d = nc.dram_tensor("out", [P, nslots * P], BF, kind="ExternalOutput")
    stat_d = nc.dram_tensor("stat", [(NB + 1) * P, P], BF)

    with tile.TileContext(nc) as tc:
        with (
            tc.tile_pool(name="consts", bufs=1) as consts,
            tc.tile_pool(name="scratch", bufs=2) as scratch,
            tc.tile_pool(name="atab", bufs=1) as atab,
            tc.tile_pool(name="btab", bufs=1) as btabp,
            tc.tile_pool(name="sstage", bufs=4) as sstagep,
            tc.tile_pool(name="stage", bufs=int(os.environ.get("STAGE_BUFS", "8"))) as stagep,
            tc.tile_pool(name="obuf", bufs=int(os.environ.get("OBUF_BUFS", "3"))) as obufp,
            tc.tile_pool(name="psB", bufs=int(os.environ.get("PSB_BUFS", "2")), space="PSUM") as psB,
            tc.tile_pool(name="psP", bufs=int(os.environ.get("PSP_BUFS", "3")), space="PSUM") as psP,
        ):
            ident = consts.tile([P, P], FP, tag="ident")
            make_identity(nc, ident[:])
            identb = consts.tile([P, P], BF, tag="identb")
            nc.vector.tensor_copy(identb[:], ident[:])
            praw = consts.tile([P, 2, P], FP, tag="praw")
            nc.sync.dma_start(praw[:], praw_d[:].rearrange("p r c -> r p c"))
            sioff = consts.tile([1, nblk], I32, tag="sioff")
            bidx = consts.tile([1, nslots], I32, tag="bidx")
            nc.sync.dma_start(sioff[:], sioff_d[:])
            nc.sync.dma_start(bidx[:], bidx_d[:])

            # ---- phase A: primitives (fp32) ----
            G, GT = _build_expm(nc, consts, psB, scratch, praw, ident)

            # ---- phase B: A2/A2T/A4/A4T doubling tables (fp32) ----
            a2 = atab.tile([P, 4, P], FP, tag="a2")
            a2t = atab.tile([P, 4, P], FP, tag="a2t")
            for m in range(4):
                ps = psB.tile([P, 512], FP, tag="psb")
                _mm(nc, ps[:, :P], GT[m & 1][:], G[m >> 1][:])   # A2[m]
                nc.vector.tensor_copy(a2[:, m, :], ps[:, :P])
                ps2 = psB.tile([P, 512], FP, tag="psb")
                _mm(nc, ps2[:, :P], G[m >> 1][:], GT[m & 1][:])  # A2T[m]
                nc.scalar.copy(a2t[:, m, :], ps2[:, :P])
            # a4/a4t feed only fp32r matmuls -> declare fp32r; the evac
            # copies round-on-write as the BIR verifier requires.
            a4 = atab.tile([P, 16, P], FR, tag="a4")
            a4t = atab.tile([P, 16, P], FR, tag="a4t")
            a2f = a2[:].rearrange("r m c -> r (m c)")
            a2tf = a2t[:].rearrange("r m c -> r (m c)")
            for a in range(4):
                ps = psB.tile([P, 512], FP, tag="psb")
                _mm(nc, ps[:], a2t[:, a, :], a2f)        # A4[a+4b] over b
                for b2 in range(4):
                    nc.vector.tensor_copy(
                        a4[:, a + 4 * b2, :], ps[:, b2 * P : (b2 + 1) * P]
                    )
                # A4T[m] = A2T[m>>2] @ A2T[m&3]; fix a=m>>2: m = 4a+b contiguous
                ps2 = psB.tile([P, 512], FP, tag="psb")
                _mm(nc, ps2[:], a2[:, a, :], a2tf)
                nc.scalar.copy(
                    a4t[:, 4 * a : 4 * a + 4, :].rearrange("r m c -> r (m c)"),
                    ps2[:],
                )

            # ---- phase C: stat table (A8^T) -> stat_d rows (bf16) ----
            a4tf = a4t[:].rearrange("r m c -> r (m c)")
            stat_v = stat_d[:].rearrange("(e r) c -> r e c", r=P)
            for g in range(16):
                for q in range(4):
                    sst = sstagep.tile([P, 4, P], BF, tag="sst")
                    ps = psB.tile([P, 512], FP, tag="psb")
                    # stat[16g + (4q+j)] = A4T[g] @ A4T[4q+j], j=0..3
                    _mm(nc, ps[:], a4[:, g, :], a4tf[:, q * 512 : (q + 1) * 512])
                    eng = nc.vector if (g + q) % 2 == 0 else nc.scalar
                    if eng is nc.vector:
                        eng.tensor_copy(sst[:], ps[:].rearrange("r (m c) -> r m c", c=P))
                    else:
                        eng.copy(sst[:], ps[:].rearrange("r (m c) -> r m c", c=P))
                    nc.sync.dma_start(
                        stat_v[:, 16 * g + 4 * q : 16 * g + 4 * q + 4, :], sst[:]
                    )
            sstI = sstagep.tile([P, 4, P], BF, tag="sst")
            nc.vector.tensor_copy(sstI[:, 0, :], ident[:])
            nc.sync.dma_start(stat_v[:, NB : NB + 1, :], sstI[:, 0:1, :])

            # ---- phase D: btab (bf16 SBUF [P, 16(q), 16(m), P]) ----
            btab = btabp.tile([P, 16, 16, P], BF, tag="btab")
            m15 = atab.tile([P, 16, P], FP, tag="m15")
            nc.vector.tensor_copy(m15[:, 1, :], ident[:])
            nc.vector.tensor_copy(m15[:, 2, :], G[0][:])
            nc.vector.tensor_copy(m15[:, 3, :], G[1][:])
            nc.scalar.copy(btab[:, 0, 0, :], ident[:])
            nc.scalar.copy(btab[:, 0, 1, :], ident[:])
            nc.scalar.copy(btab[:, 0, 2, :], G[0][:])
            nc.scalar.copy(btab[:, 0, 3, :], G[1][:])
            # entries 4..15: M(2c+b) = G_b @ M(c); strided views over (c b2)
            m15v = m15[:].rearrange("r (c b2) p -> r c b2 p", b2=2)
            btsv = btab[:, 0, :, :].rearrange("r (c b2) p -> r c b2 p", b2=2)
            for (c0, c1) in ((2, 4), (4, 8)):
                ncols = c1 - c0
                for b in range(2):
                    ps = psB.tile([P, 512], FP, tag="psb")
                    _mm(nc, ps[:, : ncols * P], GT[b][:],
                        m15[:, c0:c1, :].rearrange("r m c -> r (m c)"))
                    psv = ps[:, : ncols * P].rearrange("r (m c) -> r m c", c=P)
                    nc.vector.tensor_copy(m15v[:, c0:c1, b, :], psv)
                    nc.scalar.copy(btsv[:, c0:c1, b, :], psv)
            # fp32r copy of M(1..15) for the fp32r big phase
            m15r = atab.tile([P, 16, P], FR, tag="m15r")
            nc.vector.tensor_copy(
                m15r[:, 1:16, :].rearrange("r m c -> r (m c)"),
                m15[:, 1:16, :].rearrange("r m c -> r (m c)"),
            )
            # entries 16..255: M(16q+m) = A4(m) @ M(q), fp32r; 4 q-chunks per m
            for m in range(16):
                for (q0, q1) in ((1, 5), (5, 9), (9, 13), (13, 16)):
                    nq = q1 - q0
                    ps = psB.tile([P, 512], FP, tag="psb")
                    _mm(nc, ps[:, : nq * P], a4t[:, m, :],
                        m15r[:, q0:q1, :].rearrange("r m c -> r (m c)"))
                    psv = ps[:, : nq * P].rearrange("r (m c) -> r m c", c=P)
                    eng = nc.vector if (m + q0) % 2 == 0 else nc.scalar
                    if eng is nc.vector:
                        eng.tensor_copy(btab[:, q0:q1, m, :], psv)
                    else:
                        eng.copy(btab[:, q0:q1, m, :], psv)

            # ---- phase E: position loop ----
            btf = btab[:].rearrange("r q m p -> r (q m p)")
            base16 = n16 * 16
            with contextlib.ExitStack() as regctx:
                regs = [regctx.enter_context(nc.tensor.register(f"rb{j}"))
                        for j in range(16)]
                rs = regctx.enter_context(nc.sync.register("rs"))

                def stage_st(blk):
                    st = stagep.tile([P, P], BF, tag="st")
                    nc.sync.reg_load(rs, sioff[0:1, blk : blk + 1])
                    so = nc.sync.snap(rs, min_val=0, max_val=NB * P)
                    nc.sync.dma_start(st[:], stat_d[bass.ds(so, P), :])
                    return st

                for sb in range(nsb):
                    s0 = sb * 16
                    nc.tensor.reg_load(regs, bidx[0:1, s0 : s0 + 16])
                    offs = [
                        nc.tensor.snap(regs[j], donate=True,
                                       min_val=0, max_val=(NB - 1) * P)
                        for j in range(16)
                    ]
                    if sb < n16:
                        sts = [stage_st(sb)] * 4
                    else:
                        t4 = sb - n16
                        sts = [stage_st(n16 + 4 * t4 + g) for g in range(4)]
                    pts = [psP.tile([P, 1024], FP, tag="pp", name=f"pp{h}")
                           for h in range(2)]
                    for j in range(16):
                        g = j // 4
                        pt = pts[j // 8]
                        nc.tensor.matmul(
                            pt[:, (j % 8) * P : (j % 8 + 1) * P],
                            lhsT=sts[g][:],
                            rhs=btf[:, bass.ds(offs[j], P)],
                            start=True, stop=True,
                        )
                    ob = obufp.tile([P, 16 * P], BF, tag="ob")
                    nc.vector.tensor_copy(ob[:, : 8 * P], pts[0][:])
                    nc.scalar.copy(ob[:, 8 * P :], pts[1][:])
                    nc.gpsimd.dma_start(
                        out_d[:, s0 * P : (s0 + 16) * P], ob[:]
                    )
    nc.compile()
    return nc


def _plan_blocks(unique):
    """Pack positions into 16-blocks and 4-blocks sharing a stationary entry."""
    lo = unique & 255
    hi = unique >> 8
    ent = np.where(hi > 0, lo, IDENT_ENTRY)
    bent = np.where(hi > 0, hi, unique)  # hi==0 -> out = I @ M(pos)
    order = np.argsort(ent, kind="stable")
    es = ent[order]
    bounds = np.flatnonzero(np.r_[True, es[1:] != es[:-1], True])

    blocks16, blocks4 = [], []
    for s, e in zip(bounds[:-1], bounds[1:]):
        idxs = order[s:e]
        v = int(es[s])
        g = len(idxs)
        q0 = 0
        while g - q0 >= 16:
            blocks16.append((v, idxs[q0 : q0 + 16]))
            q0 += 16
        while q0 < g:
            blocks4.append((v, idxs[q0 : q0 + 4]))
            q0 += 4
    return blocks16, blocks4, bent


def kernel(unique, primitives_raw, identity=None, **_):
    unique = np.asarray(unique)
    praw = np.ascontiguousarray(np.asarray(primitives_raw, np.float32))

    blocks16, blocks4, bent = _plan_blocks(unique.astype(np.int64))
    n16 = -(-len(blocks16) // NCORES)
    n4 = -(-len(blocks4) // (NCORES * 4)) * 4
    while len(blocks16) < NCORES * n16:
        blocks16.append((IDENT_ENTRY, np.empty(0, np.int64)))
    while len(blocks4) < NCORES * n4:
        blocks4.append((IDENT_ENTRY, np.empty(0, np.int64)))
    nslots = n16 * 16 + n4 * 4

    slot_of_pos = np.zeros(unique.shape[0], np.int64)
    sioff = np.zeros((NCORES, n16 + n4), np.int32)
    bidx = np.zeros((NCORES, 1, nslots), np.int32)
    for i, (v, mem) in enumerate(blocks16):
        c, k = divmod(i, n16)
        sioff[c, k] = v * P
        for j, pidx in enumerate(mem):
            bidx[c, 0, k * 16 + j] = int(bent[pidx]) * P
            slot_of_pos[pidx] = c * nslots + k * 16 + j
    for i, (v, mem) in enumerate(blocks4):
        c, k = divmod(i, n4)
        sioff[c, n16 + k] = v * P
        base = n16 * 16 + k * 4
        for j, pidx in enumerate(mem):
            bidx[c, 0, base + j] = int(bent[pidx]) * P
            slot_of_pos[pidx] = c * nslots + base + j

    key = (n16, n4)
    if key not in _prog_cache:
        _prog_cache[key] = build_program(n16, n4)
    nc = _prog_cache[key]

    in_maps = [
        {
            "praw": praw,
            "sioff": np.ascontiguousarray(sioff[c].reshape(1, -1)),
            "bidx": np.ascontiguousarray(bidx[c]),
        }
        for c in range(NCORES)
    ]
    global _last_ctx
    _last_ctx = (nc, in_maps)
    res = run_bass_kernel_spmd(nc, in_maps, list(range(NCORES)))
    outs = np.concatenate(
        [
            np.asarray(res.results[c]["out"])
            .reshape(P, nslots, P)
            .transpose(1, 0, 2)
            for c in range(NCORES)
        ],
        axis=0,
    )
    return np.ascontiguousarray(outs[slot_of_pos]).astype(np.float32)


if __name__ == "__main__":
    rng = np.random.default_rng(0)
    u = rng.integers(1, 65536, 64).astype(np.int32)
    pr = rng.random((2, P, P), np.float32)
    o = kernel(u, pr)
    print(o.shape, o.dtype)


# revision 10
# speedup vs baseline: 2.6042x; 1.0560x over previous
"""Trainium2 Bass kernel for nn_BinaryPathEncoder.

Math: out[n] = prod_k W_{b_k(pos_n)}^T (product over the binary digits of
pos_n below its leading 1; W_0/W_1 = expm(herm_b), pad -> identity).

Let G_b = W_b^T = expm(-herm_b), M(h) = G_{b_0(h)} @ G_{b_1(h)} @ ...
Split pos = hi*256 + lo:
  hi >= 1:  out = A8(lo) @ M(hi)   (8 low bits all valid)
  hi == 0:  out = I @ M(pos)
Tables (per core, identical SPMD program):
  - G via scaling-squaring Taylor in fp32; G/GT stored fp32r
  - A2/A4/A4T doubling tables, M(1..15) chains: fp32r matmuls
  - stat[lo] = A8(lo)^T = A4T[lo>>4] @ A4T[lo&15] -> bf16 DRAM table
  - btab[h] = M(h): M(16q+m) = A4(m) @ M(q) -> bf16 SBUF [P, 16(q), 16(m), P]
Position loop, superblocks of 16 slots (1 block16 or 4 block4s):
  - stationary staged by dyn-offset DMA from the DRAM stat table (sync)
  - moving operands gathered from SBUF btab by dyn-offset engine copies
    split across vector/scalar/gpsimd (reg_load of index batches per engine)
  - 4 static matmuls [128,512] bf16 -> 2 PSUM [128,1024] tiles
  - evac fp32->bf16 split vector/scalar; bf16 out DMA (gpsimd issue)
Host converts bf16->fp32 and scatters slots back to input order.
"""

import contextlib
import os

import numpy as np

import concourse.bass as bass
import concourse.bacc as bacc
import concourse.mybir as mybir
import concourse.tile as tile
import concourse.tile_utils as tile_utils
tile_utils.max_sbuf_usage = 206 * 1024
from concourse.bass_utils import run_bass_kernel_spmd
from concourse.masks import make_identity

FP = mybir.dt.float32
FR = mybir.dt.float32r
BF = mybir.dt.bfloat16
I32 = mybir.dt.int32
P = 128
NCORES = 8
S_EXP = 5          # scaling-squaring: X = -H / 2^S_EXP
ORDER = 12         # Taylor order (||H||~37 -> tail ~1e-8)
NB = 256           # table entries
IDENT_ENTRY = 256  # stationary-table entry holding the identity

# gather split per 16-slot superblock: slots per engine (vector, scalar, gpsimd)
GSPLIT = tuple(int(x) for x in os.environ.get("GSPLIT", "6,4,6").split(","))
assert sum(GSPLIT) == 16
# evac split: first EVAC_DVE columns (of 2048) on vector, rest on scalar
EVAC_DVE = int(os.environ.get("EVAC_DVE", "1024"))

_prog_cache = {}
_last_ctx = None


def _mm(nc, out, lhsT, rhs):
    nc.tensor.matmul(out, lhsT=lhsT, rhs=rhs, start=True, stop=True)


def _build_expm(nc, consts, psB, scratch, praw, ident):
    """Return (G, GT) fp32r tile pairs: G_b = expm(-H_b), GT_b = G_b^T.

    Interleaves the b=0/b=1 chains to hide serial latency. Taylor recurrence
    T <- I + (X/k) @ T with pre-scaled copies of X^T, identity added on DVE.
    """
    xtj, t, u = {}, {}, {}
    for b in range(2):
        pb = praw[:, b, :]
        ps_t = psB.tile([P, 512], FP, tag="psb")
        nc.tensor.transpose(out=ps_t[:, :P], in_=pb, identity=ident[:])
        xt = scratch.tile([P, P], FP, tag=f"xt{b}")
        nc.vector.tensor_tensor(
            out=xt[:], in0=pb, in1=ps_t[:, :P], op=mybir.AluOpType.subtract
        )
        xj = consts.tile([P, ORDER, P], FP, tag=f"xtj{b}")
        for k in range(1, ORDER + 1):
            s = 1.0 / ((1 << S_EXP) * k)
            if k % 2 == 0:
                nc.vector.tensor_scalar_mul(xj[:, k - 1, :], xt[:], s)
            else:
                nc.scalar.mul(xj[:, k - 1, :], xt[:], s)
        xtj[b] = xj
        tc0 = scratch.tile([P, P], FP, tag=f"tay{b}")
        nc.vector.tensor_copy(tc0[:], ident[:])
        t[b] = tc0
    for k in range(ORDER, 0, -1):
        for b in range(2):
            ps = psB.tile([P, 512], FP, tag="psb")
            _mm(nc, ps[:, :P], xtj[b][:, k - 1, :], t[b][:])
            tn = scratch.tile([P, P], FP, tag=f"tay{b}")
            nc.vector.tensor_tensor(
                out=tn[:], in0=ps[:, :P], in1=ident[:], op=mybir.AluOpType.add
            )
            t[b] = tn
    for b in range(2):
        ps_u = psB.tile([P, 512], FP, tag="psb")
        nc.tensor.transpose(out=ps_u[:, :P], in_=t[b][:], identity=ident[:])
        uc = scratch.tile([P, P], FP, tag=f"tayu{b}")
        nc.scalar.copy(uc[:], ps_u[:, :P])
        u[b] = uc
    for _ in range(S_EXP):
        for b in range(2):
            ps1 = psB.tile([P, 512], FP, tag="psb")
            ps2 = psB.tile([P, 512], FP, tag="psb")
            _mm(nc, ps1[:, :P], u[b][:], t[b][:])   # T' = T @ T
            _mm(nc, ps2[:, :P], t[b][:], u[b][:])   # U' = (T@T)^T
            tn = scratch.tile([P, P], FP, tag=f"tay{b}")
            un = scratch.tile([P, P], FP, tag=f"tayu{b}")
            nc.vector.tensor_copy(tn[:], ps1[:, :P])
            nc.scalar.copy(un[:], ps2[:, :P])
            t[b], u[b] = tn, un
    Gs, GTs = [], []
    for b in range(2):
        g = consts.tile([P, P], FR, tag=f"g{b}")
        gt = consts.tile([P, P], FR, tag=f"gt{b}")
        nc.vector.tensor_copy(g[:], t[b][:])
        nc.scalar.copy(gt[:], u[b][:])
        Gs.append(g)
        GTs.append(gt)
    return Gs, GTs


def build_program(n16, n4):
    assert n4 % 4 == 0
    nblk = n16 + n4
    nslots = n16 * 16 + n4 * 4
    nsb = n16 + n4 // 4          # superblocks of 16 slots
    nc = bacc.Bacc("TRN2", target_bir_lowering=False, debug=False,
                   num_devices=NCORES)
    praw_d = nc.dram_tensor("praw", [2, P, P], FP, kind="ExternalInput")
    sioff_d = nc.dram_tensor("sioff", [1, nblk], I32, kind="ExternalInput")
    bidx_d = nc.dram_tensor("bidx", [1, nslots], I32, kind="ExternalInput")
    out_d = nc.dram_tensor("out", [P, nslots * P], BF, kind="ExternalOutput")
    stat_d = nc.dram_tensor("stat", [(NB + 1) * P, P], BF)

    with tile.TileContext(nc) as tc:
        with (
            tc.tile_pool(name="consts", bufs=1) as consts,
            tc.tile_pool(name="scratch", bufs=2) as scratch,
            tc.tile_pool(name="atab", bufs=1) as atab,
            tc.tile_pool(name="btab", bufs=1) as btabp,
            tc.tile_pool(name="sstage", bufs=4) as sstagep,
            tc.tile_pool(name="stage", bufs=int(os.environ.get("STAGE_BUFS", "8"))) as stagep,
            tc.tile_pool(name="mv", bufs=int(os.environ.get("MV_BUFS", "3"))) as mvp,
            tc.tile_pool(name="obuf", bufs=int(os.environ.get("OBUF_BUFS", "3"))) as obufp,
            tc.tile_pool(name="psB", bufs=int(os.environ.get("PSB_BUFS", "4")), space="PSUM") as psB,
            tc.tile_pool(name="psP", bufs=int(os.environ.get("PSP_BUFS", "2")), space="PSUM") as psP,
        ):
            ident = consts.tile([P, P], FP, tag="ident")
            make_identity(nc, ident[:])
            praw = consts.tile([P, 2, P], FP, tag="praw")
            nc.sync.dma_start(praw[:], praw_d[:].rearrange("p r c -> r p c"))
            sioff = consts.tile([1, nblk], I32, tag="sioff")
            bidx = consts.tile([1, nslots], I32, tag="bidx")
            nc.sync.dma_start(sioff[:], sioff_d[:])
            nc.sync.dma_start(bidx[:], bidx_d[:])

            # ---- phase A: primitives ----
            G, GT = _build_expm(nc, consts, psB, scratch, praw, ident)

            # ---- phase A2: M(1..15) chains (fp32r) ----
            m15 = atab.tile([P, 16, P], FR, tag="m15")
            btab = btabp.tile([P, 16, 16, P], BF, tag="btab")
            nc.vector.tensor_copy(m15[:, 1, :], ident[:])
            nc.vector.tensor_copy(m15[:, 2, :], G[0][:])
            nc.vector.tensor_copy(m15[:, 3, :], G[1][:])
            nc.scalar.copy(btab[:, 0, 0, :], ident[:])
            nc.scalar.copy(btab[:, 0, 1, :], ident[:])
            nc.scalar.copy(btab[:, 0, 2, :], G[0][:])
            nc.scalar.copy(btab[:, 0, 3, :], G[1][:])
            # entries 4..15: M(2c+b) = G_b @ M(c); strided views over (c b2)
            m15v = m15[:].rearrange("r (c b2) p -> r c b2 p", b2=2)
            btsv = btab[:, 0, :, :].rearrange("r (c b2) p -> r c b2 p", b2=2)
            for (c0, c1) in ((2, 4), (4, 8)):
                ncols = c1 - c0
                for b in range(2):
                    ps = psB.tile([P, 512], FP, tag="psb")
                    _mm(nc, ps[:, : ncols * P], GT[b][:],
                        m15[:, c0:c1, :].rearrange("r m c -> r (m c)"))
                    psv = ps[:, : ncols * P].rearrange("r (m c) -> r m c", c=P)
                    nc.vector.tensor_copy(m15v[:, c0:c1, b, :], psv)
                    nc.scalar.copy(btsv[:, c0:c1, b, :], psv)

            # ---- phase B: A2/A2T/A4/A4T doubling tables (fp32r) ----
            a2 = atab.tile([P, 4, P], FR, tag="a2")
            a2t = atab.tile([P, 4, P], FR, tag="a2t")
            for m in range(4):
                ps = psB.tile([P, 512], FP, tag="psb")
                _mm(nc, ps[:, :P], GT[m & 1][:], G[m >> 1][:])   # A2[m]
                nc.vector.tensor_copy(a2[:, m, :], ps[:, :P])
                ps2 = psB.tile([P, 512], FP, tag="psb")
                _mm(nc, ps2[:, :P], G[m >> 1][:], GT[m & 1][:])  # A2T[m]
                nc.scalar.copy(a2t[:, m, :], ps2[:, :P])
            a4 = atab.tile([P, 16, P], FR, tag="a4")
            a4t = atab.tile([P, 16, P], FR, tag="a4t")
            a2f = a2[:].rearrange("r m c -> r (m c)")
            a2tf = a2t[:].rearrange("r m c -> r (m c)")
            for a in range(4):
                ps = psB.tile([P, 512], FP, tag="psb")
                _mm(nc, ps[:], a2t[:, a, :], a2f)        # A4[a+4b] over b
                for b2 in range(4):
                    nc.vector.tensor_copy(
                        a4[:, a + 4 * b2, :], ps[:, b2 * P : (b2 + 1) * P]
                    )
                # A4T[m] = A2T[m>>2] @ A2T[m&3]; fix a=m>>2: m = 4a+b contiguous
                ps2 = psB.tile([P, 512], FP, tag="psb")
                _mm(nc, ps2[:], a2[:, a, :], a2tf)
                nc.scalar.copy(
                    a4t[:, 4 * a : 4 * a + 4, :].rearrange("r m c -> r (m c)"),
                    ps2[:],
                )

            # ---- phase C: stat table (A8^T) -> stat_d rows (bf16) ----
            a4tf = a4t[:].rearrange("r m c -> r (m c)")
            stat_v = stat_d[:].rearrange("(e r) c -> r e c", r=P)
            for g in range(16):
                for q in range(4):
                    sst = sstagep.tile([P, 4, P], BF, tag="sst")
                    ps = psB.tile([P, 512], FP, tag="psb")
                    # stat[16g + (4q+j)] = A4T[g] @ A4T[4q+j], j=0..3
                    _mm(nc, ps[:], a4[:, g, :], a4tf[:, q * 512 : (q + 1) * 512])
                    psv = ps[:].rearrange("r (m c) -> r m c", c=P)
                    if (g + q) % 2 == 0:
                        nc.vector.tensor_copy(sst[:], psv)
                    else:
                        nc.scalar.copy(sst[:], psv)
                    nc.sync.dma_start(
                        stat_v[:, 16 * g + 4 * q : 16 * g + 4 * q + 4, :], sst[:]
                    )
            sstI = sstagep.tile([P, 4, P], BF, tag="sst")
            nc.vector.tensor_copy(sstI[:, 0, :], ident[:])
            nc.sync.dma_start(stat_v[:, NB : NB + 1, :], sstI[:, 0:1, :])

            # ---- phase D: btab entries 16..255 = A4(m) @ M(q), fp32r ----
            for m in range(16):
                for (q0, q1) in ((1, 5), (5, 9), (9, 13), (13, 16)):
                    nq = q1 - q0
                    ps = psB.tile([P, 512], FP, tag="psb")
                    _mm(nc, ps[:, : nq * P], a4t[:, m, :],
                        m15[:, q0:q1, :].rearrange("r m c -> r (m c)"))
                    psv = ps[:, : nq * P].rearrange("r (m c) -> r m c", c=P)
                    if (m + q0) % 2 == 0:
                        nc.vector.tensor_copy(btab[:, q0:q1, m, :], psv)
                    else:
                        nc.scalar.copy(btab[:, q0:q1, m, :], psv)

            # ---- phase E: position loop ----
            btf = btab[:].rearrange("r q m p -> r (q m p)")
            nV, nA, nG = GSPLIT
            with contextlib.ExitStack() as regctx:
                vregs = [regctx.enter_context(nc.vector.register(f"rv{j}"))
                         for j in range(nV)]
                aregs = [regctx.enter_context(nc.scalar.register(f"ra{j}"))
                         for j in range(nA)]
                gregs = [regctx.enter_context(nc.gpsimd.register(f"rg{j}"))
                         for j in range(nG)]
                rs = regctx.enter_context(nc.sync.register("rs"))

                def stage_st(blk):
                    st = stagep.tile([P, P], BF, tag="st")
                    nc.sync.reg_load(rs, sioff[0:1, blk : blk + 1])
                    so = nc.sync.snap(rs, min_val=0, max_val=NB * P)
                    nc.sync.dma_start(st[:], stat_d[bass.ds(so, P), :])
                    return st

                def gather(eng, regs, mv, s0, j0, cnt):
                    if cnt == 0:
                        return
                    eng.reg_load(regs[:cnt], bidx[0:1, s0 + j0 : s0 + j0 + cnt])
                    for i in range(cnt):
                        off = eng.snap(regs[i], donate=True,
                                       min_val=0, max_val=(NB - 1) * P)
                        src = btf[:, bass.ds(off, P)]
                        if eng is nc.scalar:
                            eng.copy(mv[:, j0 + i, :], src)
                        else:
                            eng.tensor_copy(mv[:, j0 + i, :], src)

                for sb in range(nsb):
                    s0 = sb * 16
                    if sb < n16:
                        sts = [stage_st(sb)] * 4
                    else:
                        t4 = sb - n16
                        sts = [stage_st(n16 + 4 * t4 + g) for g in range(4)]
                    mv = mvp.tile([P, 16, P], BF, tag="mv")
                    gather(nc.vector, vregs, mv, s0, 0, nV)
                    gather(nc.scalar, aregs, mv, s0, nV, nA)
                    gather(nc.gpsimd, gregs, mv, s0, nV + nA, nG)
                    pts = [psP.tile([P, 1024], FP, tag="pp", name=f"pp{h}")
                           for h in range(2)]
                    for q in range(4):
                        _mm(nc, pts[q // 2][:, (q % 2) * 512 : (q % 2 + 1) * 512],
                            sts[q][:],
                            mv[:, 4 * q : 4 * q + 4, :].rearrange("r m c -> r (m c)"))
                    ob = obufp.tile([P, 16 * P], BF, tag="ob")
                    # vector evacs ob[:, :EVAC_DVE], scalar the rest
                    lo = min(EVAC_DVE, 1024)
                    if lo > 0:
                        nc.vector.tensor_copy(ob[:, :lo], pts[0][:, :lo])
                    if lo < 1024:
                        nc.scalar.copy(ob[:, lo:1024], pts[0][:, lo:])
                    hi = max(EVAC_DVE, 1024)
                    if hi > 1024:
                        nc.vector.tensor_copy(
                            ob[:, 1024:hi], pts[1][:, : hi - 1024])
                    if hi < 2048:
                        nc.scalar.copy(ob[:, hi:], pts[1][:, hi - 1024 :])
                    nc.gpsimd.dma_start(
                        out_d[:, s0 * P : (s0 + 16) * P], ob[:]
                    )
    nc.compile()
    return nc


def _plan_blocks(unique):
    """Pack positions into 16-blocks and 4-blocks sharing a stationary entry."""
    lo = unique & 255
    hi = unique >> 8
    ent = np.where(hi > 0, lo, IDENT_ENTRY)
    bent = np.where(hi > 0, hi, unique)  # hi==0 -> out = I @ M(pos)
    order = np.argsort(ent, kind="stable")
    es = ent[order]
    bounds = np.flatnonzero(np.r_[True, es[1:] != es[:-1], True])

    blocks16, blocks4 = [], []
    for s, e in zip(bounds[:-1], bounds[1:]):
        idxs = order[s:e]
        v = int(es[s])
        g = len(idxs)
        q0 = 0
        while g - q0 >= 16:
            blocks16.append((v, idxs[q0 : q0 + 16]))
            q0 += 16
        while q0 < g:
            blocks4.append((v, idxs[q0 : q0 + 4]))
            q0 += 4
    return blocks16, blocks4, bent


def kernel(unique, primitives_raw, identity=None, **_):
    unique = np.asarray(unique)
    praw = np.ascontiguousarray(np.asarray(primitives_raw, np.float32))

    blocks16, blocks4, bent = _plan_blocks(unique.astype(np.int64))
    n16 = -(-len(blocks16) // NCORES)
    n4 = -(-len(blocks4) // (NCORES * 4)) * 4
    while len(blocks16) < NCORES * n16:
        blocks16.append((IDENT_ENTRY, np.empty(0, np.int64)))
    while len(blocks4) < NCORES * n4:
        blocks4.append((IDENT_ENTRY, np.empty(0, np.int64)))
    nslots = n16 * 16 + n4 * 4

    slot_of_pos = np.zeros(unique.shape[0], np.int64)
    sioff = np.zeros((NCORES, n16 + n4), np.int32)
    bidx = np.zeros((NCORES, 1, nslots), np.int32)
    for i, (v, mem) in enumerate(blocks16):
        c, k = divmod(i, n16)
        sioff[c, k] = v * P
        for j, pidx in enumerate(mem):
            bidx[c, 0, k * 16 + j] = int(bent[pidx]) * P
            slot_of_pos[pidx] = c * nslots + k * 16 + j
    for i, (v, mem) in enumerate(blocks4):
        c, k = divmod(i, n4)
        sioff[c, n16 + k] = v * P
        base = n16 * 16 + k * 4
        for j, pidx in enumerate(mem):
            bidx[c, 0, base + j] = int(bent[pidx]) * P
            slot_of_pos[pidx] = c * nslots + base + j

    key = (n16, n4)
    if key not in _prog_cache:
        _prog_cache[key] = build_program(n16, n4)
    nc = _prog_cache[key]

    in_maps = [
        {
            "praw": praw,
            "sioff": np.ascontiguousarray(sioff[c].reshape(1, -1)),
            "bidx": np.ascontiguousarray(bidx[c]),
        }
        for c in range(NCORES)
    ]
    global _last_ctx
    _last_ctx = (nc, in_maps)
    res = run_bass_kernel_spmd(nc, in_maps, list(range(NCORES)))
    outs = np.concatenate(
        [
            np.asarray(res.results[c]["out"])
            .reshape(P, nslots, P)
            .transpose(1, 0, 2)
            for c in range(NCORES)
        ],
        axis=0,
    )
    return np.ascontiguousarray(outs[slot_of_pos]).astype(np.float32)


if __name__ == "__main__":
    rng = np.random.default_rng(0)
    u = rng.integers(1, 65536, 64).astype(np.int32)
    pr = rng.random((2, P, P), np.float32)
    o = kernel(u, pr)
    print(o.shape, o.dtype)
